# revision 41
# baseline (speedup 1.0000x reference)
"""Multi-head attention on 8 TRN2 NeuronCores — fused-pipeline version.

Problem: x[4,2048,768], 12 heads x 64 dim, fused QKV/attention/output
projection (softmax without 1/sqrt(dh) scaling, matching the module).

Sharding: 8 cores = 4 batches x 2 head-groups (6 heads = 3 pairs each).

Differences vs the two-phase baseline (which ran QKV projection serially
for ~105us with the scalar engine idle, then an ACT-bound attention
phase):
  * Single fused pipeline: only q/k for the first l-block are projected
    up front (a j-outer wave that tracks the input DMA stream); all other
    projection work (q/k for later l-blocks, all of v) plus the output
    projection run as "aux" chains interleaved into the attention loop's
    spare PE slots, so exp starts at ~18us and the scalar engine stays
    saturated for the whole kernel.
  * S-matmuls are row-packed: the two heads of a pair run as two K=64
    matmuls in different PE row-groups (tile_position (0,0)/(64,0)),
    which the PE executes concurrently -- no zero-padded K=128 trick, no
    duplicated k tiles.
  * exp output (e_t) and v' are bf16: halves their SBUF footprint and
    lets the PV matmuls stream bf16.  PV keeps the ones-column trick
    (stationary [128,65], row 64 of v' = 1) so the softmax denominator
    falls out of the PV accumulation for free.
Host sums the two half-head partial outputs per batch and adds the
constant einsum(bv, Wd) + bd term (softmax rows sum to 1).
"""

import numpy as np
from contextlib import ExitStack

from ml_dtypes import bfloat16

import concourse.bass as bass
from concourse import bacc, tile, mybir
from concourse.bass_utils import run_bass_kernel_spmd

F32 = mybir.dt.float32
F32R = mybir.dt.float32r
BF16 = mybir.dt.bfloat16
FP16 = mybir.dt.float16
I16 = mybir.dt.int16
EXP = mybir.ActivationFunctionType.Exp

B, L, DM, H, DH = 4, 2048, 768, 12, 64
NCORES = 8
HPC = H // 2          # heads per core
HD = HPC * DH         # 384 head-dims per core
MCH = DM // 128       # 6 contraction chunks over model dim
CCH = HD // 128       # 3 partition chunks over per-core head dims = head pairs
LB = 512              # l (query) block
NLB = L // LB         # 4
LCH = L // 128        # 16 key chunks
DEPTH = 4             # chunk-slots of PV deferral behind the S/exp stream

PAIR_S = True         # row-packed K=64 S-matmul pairs (vs zero-padded K=128)
SIM_SAFE = False      # initialize don't-care regions so CoreSim can run

# exp routing: most key-chunks exponentiate on the scalar engine (exact
# spline exp); a few run as single-instruction Schraudolph bf16-exp on the
# otherwise-idle vector/gpsimd engines (bits16 = round(x*128/ln2 + b), fp32->
# int16 convert-on-write, bitcast to bf16).  ~4% per-weight shape error on
# those chunks, smoothed by the softmax average.
# (GPSIMD cannot read PSUM, so only the vector engine can take exp chunks)
ROUTE = {2: "D", 5: "D", 10: "D", 13: "D"}
EXPA = 184.66496239727872   # 128 / ln 2
EXPB = 16255.3              # 127*128 + sigma (sigma=-0.7)
MULT = mybir.AluOpType.mult
ADD = mybir.AluOpType.add

_CACHE = {}


def _build():
    nc = bacc.Bacc("TRN2", target_bir_lowering=False, debug=False,
                   num_devices=NCORES)
    # fp16 keeps a 10-bit mantissa (f32r/TF32-class accuracy for the score
    # path) at 2-byte width; e_t needs bf16's exponent range (exp values up
    # to ~e^50).  Mixed fp16-stationary x bf16-moving matmuls are fine.
    ET = BF16
    QK = FP16
    WD = FP16
    VT = FP16

    xt_d = nc.dram_tensor("xt", [128, NLB, MCH, LB], WD,
                          kind="ExternalInput").ap()
    wq_d = nc.dram_tensor("wq", [128, MCH, HD], WD, kind="ExternalInput").ap()
    wk_d = nc.dram_tensor("wk", [128, MCH, HD], WD, kind="ExternalInput").ap()
    wv_d = nc.dram_tensor("wv", [128, MCH, HD], WD, kind="ExternalInput").ap()
    wd_d = nc.dram_tensor("wd", [128, CCH, DM], WD, kind="ExternalInput").ap()
    bq_d = nc.dram_tensor("bq", [128, CCH], F32, kind="ExternalInput").ap()
    bk_d = nc.dram_tensor("bk", [128, CCH], F32, kind="ExternalInput").ap()
    ones_d = nc.dram_tensor("ones", [128, LCH * HPC], VT,
                            kind="ExternalInput").ap()
    out_d = nc.dram_tensor("outt", [NLB, 128, MCH, LB], F32,
                           kind="ExternalOutput").ap()

    with tile.TileContext(nc) as tc, ExitStack() as ctx:
        persist = ctx.enter_context(tc.tile_pool(name="persist", bufs=1))
        # q/k transposed, pair-major: partitions of chunk c = head 2c dims
        # 0..63 then head 2c+1 dims 0..63
        qt = persist.tile([128, CCH, L], QK)
        kc = persist.tile([128, CCH, L], QK)
        vsb = persist.tile([128, LCH, HPC, DH + 1], VT)
        wd_sb = persist.tile([128, CCH, DM], WD)
        bq_sb = persist.tile([128, CCH], F32)
        bk_sb = persist.tile([128, CCH], F32)
        scratch = persist.tile([1, 4], F32)

        xw = ctx.enter_context(tc.tile_pool(name="xw", bufs=1))
        xt_sb = xw.tile([128, NLB, MCH, LB], WD)
        wq_sb = xw.tile([128, MCH, HD], WD)
        wk_sb = xw.tile([128, MCH, HD], WD)
        wv_sb = xw.tile([128, MCH, HD], WD)

        ptpool = ctx.enter_context(tc.tile_pool(name="ptpool", bufs=1))
        pt = ptpool.tile([128, CCH, L], QK)

        # warm the exp table set on ACT while DMAs stream (one-time ~2.7us)
        nc.vector.memset(scratch, 0.0)
        nc.scalar.activation(scratch, scratch, EXP)

        # small high-priority loads first on the gpsimd queue
        nc.gpsimd.dma_start(bq_sb, bq_d)
        nc.gpsimd.dma_start(bk_sb, bk_d)
        nc.gpsimd.dma_start(
            vsb[:, :, :, DH],
            ones_d.rearrange("p (i h) -> p i h", i=LCH),
        )

        # critical stream: few BIG contiguous transfers (descriptor-size and
        # trigger-count bound the head, not bandwidth), in need-by order
        # across the three DMA-capable queues.
        nc.sync.dma_start(wq_sb, wq_d)
        nc.scalar.dma_start(wk_sb, wk_d)
        nc.gpsimd.dma_start(xt_sb[:, 0], xt_d[:, 0])
        nc.sync.dma_start(wv_sb, wv_d)
        nc.scalar.dma_start(xt_sb[:, 1], xt_d[:, 1])
        nc.sync.dma_start(xt_sb[:, 2], xt_d[:, 2])
        nc.scalar.dma_start(xt_sb[:, 3], xt_d[:, 3])
        # output-projection weights are not needed until ~mid-kernel
        nc.gpsimd.dma_start(wd_sb, wd_d)

        # ---- prologue: q/k projections for lb=0, j-outer wave over 6
        # concurrent accumulators so the matmuls track the DMA stream ----
        with ExitStack() as pro:
            pro_ps = pro.enter_context(
                tc.tile_pool(name="pro_ps", bufs=1, space="PSUM"))
            accs = {}
            for kind in ("q", "k"):
                for c in range(CCH):
                    accs[kind, c] = pro_ps.tile(
                        [128, LB], F32, name=f"pro_{kind}{c}",
                        tag=f"pro_{kind}{c}")
            for j in range(MCH):
                for kind, w_sb in (("q", wq_sb), ("k", wk_sb)):
                    for c in range(CCH):
                        nc.tensor.matmul(
                            accs[kind, c],
                            w_sb[:, j, c * 128:(c + 1) * 128],
                            xt_sb[:, 0, j, :],
                            start=(j == 0), stop=(j == MCH - 1),
                            skip_group_check=True)
            for c in range(CCH):
                nc.vector.tensor_scalar_add(
                    qt[:, c, 0:LB], accs["q", c], bq_sb[:, c:c + 1])
                nc.vector.tensor_scalar_add(
                    kc[:, c, 0:LB], accs["k", c], bk_sb[:, c:c + 1])

        # ---- fused attention + interleaved projection/outproj chains ----
        with ExitStack() as p2:
            s_ps = p2.enter_context(
                tc.tile_pool(name="s_ps", bufs=2, space="PSUM"))
            pv_ps = p2.enter_context(
                tc.tile_pool(name="pv_ps", bufs=2, space="PSUM"))
            aux_ps = p2.enter_context(
                tc.tile_pool(name="aux_ps", bufs=2, space="PSUM"))
            et_pool = p2.enter_context(tc.tile_pool(name="et", bufs=8))
            small = p2.enter_context(tc.tile_pool(name="small", bufs=3))
            dram = p2.enter_context(
                tc.tile_pool(name="dram", bufs=2, space="DRAM"))
            stage = p2.enter_context(tc.tile_pool(name="stage", bufs=3))

            def mk_qk_chain(w_sb, b_sb, dst, c, lb):
                def emit():
                    ps = aux_ps.tile([128, LB], F32, tag="aux")
                    for j in range(MCH):
                        nc.tensor.matmul(
                            ps,
                            w_sb[:, j, c * 128:(c + 1) * 128],
                            xt_sb[:, lb, j, :],
                            start=(j == 0), stop=(j == MCH - 1),
                            skip_group_check=True)
                    nc.vector.tensor_scalar_add(
                        dst[:, c, lb * LB:(lb + 1) * LB], ps, b_sb[:, c:c + 1])
                return emit

            def mk_v_chain(i):
                def emit():
                    ps = aux_ps.tile([128, LB], F32, tag="aux")
                    lbb, sub = divmod(i, NLB)
                    for j in range(MCH):
                        nc.tensor.matmul(
                            ps[:, 0:HD],
                            xt_sb[:, lbb, j, sub * 128:(sub + 1) * 128],
                            wv_sb[:, j, :],
                            start=(j == 0), stop=(j == MCH - 1),
                            skip_group_check=True)
                    nc.vector.tensor_copy(
                        vsb[:, i, :, 0:DH],
                        ps[:, 0:HD].rearrange("p (h d) -> p h d", h=HPC))
                return emit

            def mk_outproj(lb, mj):
                def emit():
                    lsl = slice(lb * LB, (lb + 1) * LB)
                    ps = aux_ps.tile([128, LB], F32, tag="aux")
                    for c_ in range(CCH):
                        nc.tensor.matmul(
                            ps,
                            wd_sb[:, c_, mj * 128:(mj + 1) * 128],
                            pt[:, c_, lsl],
                            start=(c_ == 0), stop=(c_ == CCH - 1))
                    o_sb = stage.tile([128, LB], F32, name="o_sb", tag="o")
                    nc.vector.tensor_copy(o_sb, ps)
                    nc.sync.dma_start(out_d[lb, :, mj, :], o_sb)
                return emit

            def mk_pv2(ph, p, i, e_ap):
                def emit():
                    if i == 0:
                        ph["A"] = pv_ps.tile([128, LB], F32, name="accA",
                                             tag="acc")
                        ph["B"] = pv_ps.tile([128, LB], F32, name="accB",
                                             tag="acc")
                    for key, h, t in (("A", 2 * p, 0), ("B", 2 * p + 1, 1)):
                        nc.tensor.matmul(
                            ph[key][0:DH + 1, :],
                            vsb[:, i, h, :],
                            e_ap[:, t, :],
                            start=(i == 0), stop=(i == LCH - 1),
                            skip_group_check=True)
                return emit

            fins_done = [0]

            def mk_fin(ph, key, h, lsl):
                # normalize: pt rows = acc[0:64] / acc[64] (+v-bias on host)
                def emit():
                    acc = ph.pop(key)
                    fins_done[0] += 1
                    p0 = (h % 2) * 64
                    hc = h // 2
                    # early psum drain: single copy of PV rows + denominator
                    # row releases the bank for the next block's accumulator
                    pvs = small.tile([128, LB], F32)
                    if SIM_SAFE:
                        nc.vector.memset(pvs[DH:128, :], 1.0)
                    nc.vector.tensor_copy(pvs[0:DH + 1, :], acc[0:DH + 1, :])
                    rec = small.tile([128, LB], F32)
                    # full-tile: the custom-DVE op silently no-ops on
                    # partition slices; rows other than 64 are don't-care
                    nc.vector.reciprocal_approx_fast(rec, pvs)
                    rec_dr = dram.tile([1, LB], F32)
                    nc.sync.dma_start(rec_dr, rec[64:65, :])
                    rcb = small.tile([64, LB], F32)
                    nc.sync.dma_start(rcb, rec_dr.broadcast_to([64, LB]))
                    nc.vector.tensor_mul(
                        pt[p0:p0 + DH, hc, lsl], pvs[0:DH, :], rcb)
                return emit

            pending = []

            def flush(keep):
                while len(pending) > keep:
                    pending.pop(0)()

            # Aux work in need-by order.  Keys are global: every block scans
            # all 2048 key positions, so kc chunks (c, lb) are needed at slot
            # 16c + 4lb of the block sequence and v(i) by its PV at ~i+DEPTH;
            # q for (c, lb) only at block 3lb+c.  The first block stalls
            # per-chunk on the k/v chains as their xt DMA lands, which is
            # exactly the DMA-arrival rate anyway.
            # v(0..3) are emitted before the block loop (their xt/wv arrive
            # with the prologue stream); the rest weave into the slots below.
            for i in range(4):
                mk_v_chain(i)()
            aux_items = []
            for i in range(4, LCH):
                aux_items.append((i + 3, 0, mk_v_chain(i)))
            for c in range(CCH):
                for lb in range(1, NLB):
                    aux_items.append(
                        (16 * c + 4 * lb, -1,
                         mk_qk_chain(wk_sb, bk_sb, kc, c, lb)))
            for lb in range(1, NLB):
                for c in range(CCH):
                    aux_items.append(
                        (16 * (3 * lb + c), 0,
                         mk_qk_chain(wq_sb, bq_sb, qt, c, lb)))
            aux_items.sort(key=lambda t: (t[0], t[1]))
            aux_items = [(s, fn) for s, _, fn in aux_items]
            LEAD = 6
            op_todo = []  # (ready_fin_count, emit)

            blk = 0
            gslot = 0
            for lb in range(NLB):
                lsl = slice(lb * LB, (lb + 1) * LB)
                for p in range(CCH):
                    ph = {}
                    for i in range(LCH):
                        ksl = slice(i * 128, (i + 1) * 128)
                        s_pair = s_ps.tile([128, 2, LB], F32, tag="s")
                        if PAIR_S:
                            # two K=64 matmuls in different PE row-groups:
                            # tile_position (0,0) / (64,0), run concurrently
                            nc.tensor.matmul(
                                s_pair[:, 0, :], kc[0:64, p, ksl],
                                qt[0:64, p, lsl],
                                start=True, stop=True, skip_group_check=True)
                            nc.tensor.matmul(
                                s_pair[:, 1, :], kc[64:128, p, ksl],
                                qt[64:128, p, lsl],
                                start=True, stop=True, skip_group_check=True)
                        else:
                            nc.tensor.matmul(
                                s_pair[:, 0, :], kc[:, p, ksl],
                                qt[:, p, lsl],
                                start=True, stop=True, skip_group_check=True)
                            nc.tensor.matmul(
                                s_pair[:, 1, :], kc[:, p, ksl],
                                qt[:, p, lsl],
                                start=True, stop=True, skip_group_check=True)
                        eng = ROUTE.get(i)
                        if eng is None:
                            e_t = et_pool.tile([128, 2, LB], ET, tag="e")
                            nc.scalar.activation(
                                e_t[:, 0:2, :], s_pair[:, 0:2, :], EXP)
                            e_ap = e_t
                        else:
                            # Schraudolph: write a real int16 tile (writes
                            # through a bitcast AP silently no-op), read it
                            # back as bf16 via a whole-tile bitcast
                            e_i = et_pool.tile([128, 2, LB], I16,
                                               name="e_t", tag="e")
                            (nc.vector if eng == "D"
                             else nc.gpsimd).tensor_scalar(
                                e_i[:, 0:2, :], s_pair[:, 0:2, :],
                                EXPA, EXPB, op0=MULT, op1=ADD)
                            e_ap = e_i.bitcast(BF16)
                        pending.append(mk_pv2(ph, p, i, e_ap))
                        # aux before flush: a v-chain must enter the PE
                        # stream before the PV that reads its output
                        budget = 2 if (aux_items
                                       and aux_items[0][0] <= gslot) else 1
                        emitted = 0
                        while (aux_items and emitted < budget
                                and aux_items[0][0] - LEAD <= gslot):
                            aux_items.pop(0)[1]()
                            emitted += 1
                        if (not emitted and op_todo
                                and op_todo[0][0] + 2 <= fins_done[0]):
                            op_todo.pop(0)[1]()
                        last_blk = (blk == NLB * CCH - 1)
                        flush(1 if last_blk else DEPTH)
                        gslot += 1
                    pending.append(mk_fin(ph, "A", 2 * p, lsl))
                    pending.append(mk_fin(ph, "B", 2 * p + 1, lsl))
                    blk += 1
                for mj in range(MCH):
                    op_todo.append(((lb + 1) * HPC, mk_outproj(lb, mj)))
            flush(0)
            for _, fn in aux_items:
                fn()
            for _, fn in op_todo:
                fn()

    nc.compile()
    return nc


def _in_maps(x, Wq, bq, Wk, bk, Wv, bv, Wd, bd):
    wd_np = np.float16
    ones = np.ones((128, LCH * HPC), np.float16)
    maps = []
    for c in range(NCORES):
        b = c // 2
        hs = (c % 2) * HPC
        xt = np.ascontiguousarray(
            x[b].T.reshape(MCH, 128, L).transpose(1, 0, 2)
            .reshape(128, MCH, NLB, LB).transpose(0, 2, 1, 3)
        ).astype(wd_np)
        wq = np.ascontiguousarray(
            Wq[:, hs:hs + HPC, :].reshape(DM, HD)
            .reshape(MCH, 128, HD).transpose(1, 0, 2)).astype(wd_np)
        wk = np.ascontiguousarray(
            Wk[:, hs:hs + HPC, :].reshape(DM, HD)
            .reshape(MCH, 128, HD).transpose(1, 0, 2)).astype(wd_np)
        wv = np.ascontiguousarray(
            Wv[:, hs:hs + HPC, :].reshape(DM, HD)
            .reshape(MCH, 128, HD).transpose(1, 0, 2)).astype(wd_np)
        wd = np.ascontiguousarray(
            Wd[hs:hs + HPC].reshape(HD, DM)
            .reshape(CCH, 128, DM).transpose(1, 0, 2)).astype(wd_np)
        bqs = np.ascontiguousarray(
            bq[hs:hs + HPC].reshape(HD).reshape(CCH, 128).T)
        bks = np.ascontiguousarray(
            bk[hs:hs + HPC].reshape(HD).reshape(CCH, 128).T)
        maps.append({"xt": xt, "wq": wq, "wk": wk, "wv": wv, "wd": wd,
                     "bq": bqs, "bk": bks, "ones": ones})
    return maps


def run(x, Wq, bq, Wk, bk, Wv, bv, Wd, bd, trace=False):
    if "nc" not in _CACHE:
        _CACHE["nc"] = _build()
    nc = _CACHE["nc"]
    maps = _in_maps(x, Wq, bq, Wk, bk, Wv, bv, Wd, bd)
    r = run_bass_kernel_spmd(nc, maps, list(range(NCORES)), trace=trace)
    out = np.zeros((B, L, DM), np.float32)
    for c in range(NCORES):
        b = c // 2
        arr = r.results[c]["outt"]  # [lb, p, mj, t]
        out[b] += arr.transpose(2, 1, 0, 3).reshape(DM, L).T
    const = bd.astype(np.float64) + np.einsum(
        "hd,hdm->m", bv.astype(np.float64),
        Wd.reshape(H, DH, DM).astype(np.float64))
    out += const.astype(np.float32).reshape(1, 1, DM)
    return out, r


def kernel(x, Wq, bq, Wk, bk, Wv, bv, Wd, bd):
    args = [np.asarray(a, dtype=np.float32)
            for a in (x, Wq, bq, Wk, bk, Wv, bv, Wd, bd)]
    out, _ = run(*args)
    return out
